# revision 19
# baseline (speedup 1.0000x reference)
"""Tensor-parallel attention kernel for trn2 (8 cores).

TP over heads (2/core) for QKV + attention; AllToAll reshards attention
output to seq-parallel; output projection seq-sharded (each core owns
256 output rows); host concatenates.

v4 layout/engine plan:
- x, wq/wk/wv, q/k/v in bf16 (halves HBM + SBUF); mask shipped as
  exp(mask) f16 so the mask "add" becomes a 2x-rate DVE f16 multiply
  after the exp.
- Phase A: per 512-col chunk, three pair-passes (q0q1 / k0k1 / v0v1)
  reuse each streamed x tile from SBUF, so x is read once and the PE
  never waits on x DMA. x rides the sync/scalar HW queues exclusively;
  w, wo and mask prefetches ride the gpsimd queue so they never
  stall x.
- V transposed to [kv, dh] on the PE (identity matmul), deferred one
  chunk so the transposes never stall. (DMA-XBAR transposes complete
  out-of-order against regular copies on the shared HW DMA queues and
  corrupt results intermittently — do not use them here.)
- Attention q-columns are processed in a core-striped order (the
  mask is host-permuted to match): each half-pair covers 128 columns
  of EVERY core's 256-row output block, so an AllToAll of that half
  can launch as soon as the half finishes -> 4 small collectives that
  pipeline behind compute instead of 2 exposed ones.
- Attention per kv-tile: 2 score matmuls -> exp on Act -> DVE multiply
  by exp(mask) -> PV matmuls. Softmax denom: chunk B summed on PE
  (ones matmul accumulate), chunk A accumulated on DVE in f16 and
  reduced once. 1/sums via fast approx reciprocal. Per-pair epilogues
  are deferred into the next pair's pipeline so the PE never waits on
  the Act->PE sums round-trip.
- wo f16 fully prefetched into SBUF during phase A; output matmuls
  grouped by q-half so the last collective hides behind ready work;
  y written f16, widened on host.
"""
import math
import numpy as np

import concourse.bass as bass
import concourse.mybir as mybir
import concourse.tile as tile
from concourse import bacc

f32 = mybir.dt.float32
f32r = mybir.dt.float32r
f16 = mybir.dt.float16
bf16 = mybir.dt.bfloat16

P = 128
S = 2048
D = 2048
HD = 128
NH = 2          # heads per core
W = 8           # cores
QS = S // W     # 256 output rows per core
DT = D // P     # 16 contraction tiles
KT = S // P     # 16 kv tiles
CH = 4          # phase-A column chunks of 512

Exp = mybir.ActivationFunctionType.Exp
ADD = mybir.AluOpType.add
MULT = mybir.AluOpType.mult


def build():
    nc = bacc.Bacc("TRN2", target_bir_lowering=False, debug=False, num_devices=W)

    xt = nc.dram_tensor("xt", [D, S], bf16, kind="ExternalInput").ap()
    wq_t = nc.dram_tensor("wq_t", [NH, P, DT, HD], bf16, kind="ExternalInput").ap()
    wk_t = nc.dram_tensor("wk_t", [NH, P, DT, HD], bf16, kind="ExternalInput").ap()
    wv_t = nc.dram_tensor("wv_t", [NH, P, DT, HD], bf16, kind="ExternalInput").ap()
    # emask_t is host-permuted along q: column p*1024 + j*128 + i holds
    # exp(mask)[kv, j*256 + p*128 + i]
    emask_t = nc.dram_tensor("emask_t", [NH, S, S], f16, kind="ExternalInput").ap()
    wo_t = nc.dram_tensor("wo_t", [D, D], f16, kind="ExternalInput").ap()
    y = nc.dram_tensor("y", [QS, D], f16, kind="ExternalOutput").ap()

    wqkv = {"q": wq_t, "k": wk_t, "v": wv_t}

    with tile.TileContext(nc) as tc:
        persist = tc.alloc_tile_pool(name="persist", bufs=1)
        consts = tc.alloc_tile_pool(name="consts", bufs=1)
        maskp = tc.alloc_tile_pool(name="maskp", bufs=2)
        dram = tc.alloc_tile_pool(name="dram", bufs=1, space="DRAM")

        from concourse.masks import make_identity
        ident_f = consts.tile([P, P], f32, name="ident_f")
        make_identity(nc, ident_f[:])
        ident = consts.tile([P, P], bf16, name="ident")
        nc.vector.tensor_copy(out=ident[:], in_=ident_f[:])
        ones_cf = consts.tile([P, 1], f32, name="ones_cf")
        nc.gpsimd.memset(ones_cf[:], 1.0)
        ones_col = consts.tile([P, 1], f16, name="ones_col")
        nc.vector.tensor_copy(out=ones_col[:], in_=ones_cf[:])
        ones_rf = consts.tile([1, P], f32, name="ones_rf")
        nc.gpsimd.memset(ones_rf[:], 1.0)
        ones_row = consts.tile([1, P], f16, name="ones_row")
        nc.vector.tensor_copy(out=ones_row[:], in_=ones_rf[:])

        qT = [persist.tile([P, S], bf16, name=f"qT{b}") for b in range(NH)]
        kT = [persist.tile([P, S], bf16, name=f"kT{b}") for b in range(NH)]
        vT = [persist.tile([P, S], bf16, name=f"vT{b}") for b in range(NH)]
        v_sb = [persist.tile([P, KT, P], bf16, name=f"v_sb{b}") for b in range(NH)]
        # wo rows h*128:(h+1)*128 live at wo_all[:, h, :]
        wo_all = persist.tile([P, DT, D], f16, name="wo_all")

        # one AllToAll per (head, q-half): [W, dh, 128]
        a2a_in = [[dram.tile([W, P, P], f16, name=f"a2a_in{b}_{p}")
                   for p in range(2)] for b in range(NH)]
        a2a_out = [[dram.tile([W, P, P], f16, name=f"a2a_out{b}_{p}")
                    for p in range(2)] for b in range(NH)]

        def em_dma(b, pair, t):
            emt = maskp.tile([P, 1024], f16, name=f"em{t}", tag=f"em{t}", bufs=2)
            nc.gpsimd.dma_start(
                emt[:], emask_t[b, t * P:(t + 1) * P,
                                pair * 1024:(pair + 1) * 1024])
            return emt

        # ---------------- Phase A: projections ----------------
        psA = tc.alloc_tile_pool(name="psA", bufs=4, space="PSUM")
        wpool = tc.alloc_tile_pool(name="wpool", bufs=1)
        xpool = tc.alloc_tile_pool(name="xpool", bufs=1)

        kinds6 = [("q", 0), ("q", 1), ("k", 0), ("k", 1), ("v", 0), ("v", 1)]
        dests = {"q": qT, "k": kT, "v": vT}
        copy_eng = {("q", 0): "act", ("q", 1): "act",
                    ("k", 0): "dve", ("k", 1): "dve",
                    ("v", 0): "act", ("v", 1): "dve"}

        with nc.named_scope("A"):
            wtiles = {}
            for kind, b in kinds6:
                wt = wpool.tile([P, DT, HD], bf16, name=f"w_{kind}{b}",
                                tag=f"w_{kind}{b}")
                if kind == "q":
                    # split so the first matmuls only wait on 256KB
                    nc.gpsimd.dma_start(wt[:, 0:4, :], wqkv[kind][b][:, 0:4, :])
                    nc.gpsimd.dma_start(wt[:, 4:16, :],
                                        wqkv[kind][b][:, 4:16, :])
                else:
                    nc.gpsimd.dma_start(wt[:], wqkv[kind][b])
                wtiles[(kind, b)] = wt

            def do_transposes(c):
                for b in range(NH):
                    for t in range(4 * c, 4 * c + 4):
                        pst = psA.tile([P, P], bf16, name="pst", tag="pst",
                                       bufs=2)
                        nc.tensor.transpose(pst[:],
                                            vT[b][:, t * P:(t + 1) * P],
                                            ident[:])
                        nc.vector.tensor_copy(out=v_sb[b][:, t, :],
                                              in_=pst[:])

            def do_chunk(c):
                col0 = c * 512
                xts = []
                for t in range(DT):
                    xtile = xpool.tile([P, 512], bf16, name=f"xt{t}",
                                       tag=f"xt{t}")
                    (nc.sync if t % 2 == 0 else nc.scalar).dma_start(
                        xtile[:], xt[t * P:(t + 1) * P, col0:col0 + 512])
                    xts.append(xtile)
                if c > 0:
                    # previous chunk's V transposes: their PSUM->SBUF v
                    # copies have had a full chunk to land, so no PE stall
                    do_transposes(c - 1)
                for kind in ("q", "k", "v"):
                    pss = {}
                    for b in range(NH):
                        pss[b] = psA.tile([P, 512], f32,
                                          name=f"psa_{kind}{b}", tag="psa")
                    for t in range(DT):
                        for b in range(NH):
                            nc.tensor.matmul(pss[b][:],
                                             wtiles[(kind, b)][:, t, :],
                                             xts[t][:],
                                             start=(t == 0), stop=(t == DT - 1))
                    for b in range(NH):
                        dst = dests[kind][b][:, col0:col0 + 512]
                        if copy_eng[(kind, b)] == "act":
                            nc.scalar.copy(dst, pss[b][:])
                        else:
                            nc.vector.tensor_copy(out=dst, in_=pss[b][:])
            for c in range(CH):
                do_chunk(c)
                if c == 1:
                    # first attention pair's masks stream during late A,
                    # BEFORE any wo prefetch hits the gpsimd ring
                    em_next = [em_dma(0, 0, t) for t in range(KT)]
            do_transposes(CH - 1)
            # wo rows 0-7 at A's end (gpsimd ring is past the masks; sync
            # ring is past x); rows 8-15 stream inside the attention pairs
            for h in range(0, 4):
                nc.gpsimd.dma_start(wo_all[:, h, :],
                                    wo_t[h * P:(h + 1) * P, :])
            for h in range(4, 8):
                nc.sync.dma_start(wo_all[:, h, :],
                                  wo_t[h * P:(h + 1) * P, :])

        xpool.release()
        wpool.release()
        psA.release()

        # ---------------- Phase B: attention ----------------
        psS = tc.alloc_tile_pool(name="psS", bufs=1, space="PSUM")
        psPV = tc.alloc_tile_pool(name="psPV", bufs=1, space="PSUM")
        espool = tc.alloc_tile_pool(name="espool", bufs=3)
        prpool = tc.alloc_tile_pool(name="prpool", bufs=3)
        smallp = tc.alloc_tile_pool(name="smallp", bufs=2)
        atpool = tc.alloc_tile_pool(name="atpool", bufs=1)

        at_tiles = {}
        pairs = [(b, p) for b in range(NH) for p in range(2)]
        pending = [None]  # deferred epilogue closure

        for pi, (b, pair) in enumerate(pairs):
          with nc.named_scope(f"B{b}{pair}"):
            emts = em_next
            if pi + 1 < len(pairs):
                nb, np_ = pairs[pi + 1]
                em_next = [em_dma(nb, np_, t) for t in range(KT)]
            # core-striped q columns: new col p*1024 + j*128 + i reads
            # qT col j*256 + p*128 + i  (emask already host-permuted)
            qr = qT[b][:].rearrange("d (j i) -> d j i", i=QS)
            qA = qr[:, 0:4, pair * P:(pair + 1) * P]
            qB = qr[:, 4:8, pair * P:(pair + 1) * P]
            pv0 = psPV.tile([P, 512], f32, name="pv0", tag="pv0")
            pv1 = psPV.tile([P, 512], f32, name="pv1", tag="pv1")
            sums_ps = psS.tile([1, 512], f32, name="sums_ps",
                               tag="sums", bufs=1)
            sacc = smallp.tile([P, 512], f16, name="sacc", tag="sacc")
            PIPE = 3
            pq = {}
            for tt in range(KT + PIPE):
                if tt < KT:
                    t = tt
                    ktile = kT[b][:, t * P:(t + 1) * P]
                    ps_sA = psS.tile([P, 512], f32, name="ps_sA",
                                     tag="ps_s", bufs=4)
                    nc.tensor.matmul(ps_sA[:], ktile, qA,
                                     start=True, stop=True)
                    ps_sB = psS.tile([P, 512], f32, name="ps_sB",
                                     tag="ps_s", bufs=4)
                    nc.tensor.matmul(ps_sB[:], ktile, qB,
                                     start=True, stop=True)
                    esA = espool.tile([P, 512], f16, name="esA", tag="es", bufs=4)
                    nc.scalar.activation(esA[:], ps_sA[:], Exp)
                    esB = espool.tile([P, 512], f16, name="esB", tag="es", bufs=4)
                    nc.scalar.activation(esB[:], ps_sB[:], Exp)
                    prA = prpool.tile([P, 512], f16, name="prA", tag="pr", bufs=5)
                    nc.vector.tensor_tensor(out=prA[:], in0=esA[:],
                                            in1=emts[t][:, 0:512], op=MULT)
                    prB = prpool.tile([P, 512], f16, name="prB", tag="pr", bufs=5)
                    nc.vector.tensor_tensor(out=prB[:], in0=esB[:],
                                            in1=emts[t][:, 512:1024], op=MULT)
                    pq[t] = (prA, prB)
                if tt == 1 and pending[0] is not None:
                    # previous pair's deferred epilogue slots in here so
                    # its PE work overlaps this pair's pipeline fill
                    pending[0]()
                    pending[0] = None
                if tt >= PIPE:
                    t = tt - PIPE
                    prA, prB = pq.pop(t)
                    vtile = v_sb[b][:, t, :]
                    nc.tensor.matmul(pv0[:], vtile, prA[:],
                                     start=(t == 0), stop=(t == KT - 1))
                    nc.tensor.matmul(pv1[:], vtile, prB[:],
                                     start=(t == 0), stop=(t == KT - 1))
                    nc.tensor.matmul(sums_ps[:], ones_col[:], prB[:],
                                     start=(t == 0), stop=(t == KT - 1))
                    if t == 0:
                        nc.vector.tensor_copy(out=sacc[:], in_=prA[:])
                    else:
                        nc.vector.tensor_tensor(out=sacc[:], in0=sacc[:],
                                                in1=prA[:], op=ADD)
            if pi < 2:
                # wo rows 8-15 stream during the first two attention
                # pairs (port is otherwise just masks here)
                for h in range(8 + 4 * pi, 12 + 4 * pi):
                    nc.gpsimd.dma_start(wo_all[:, h, :],
                                        wo_t[h * P:(h + 1) * P, :])
            # part 1 of epilogue: fold chunk-A sums, copy both to SBUF
            sumsA_ps = psS.tile([1, 512], f32, name="sumsA_ps",
                                tag="sumsA", bufs=1)
            nc.tensor.matmul(sumsA_ps[:], ones_col[:], sacc[:],
                             start=True, stop=True)
            sums_sb = smallp.tile([1, 1024], f16, name="sums_sb",
                                  tag="sums_sb")
            nc.scalar.copy(sums_sb[:, 0:512], sumsA_ps[:])
            nc.scalar.copy(sums_sb[:, 512:1024], sums_ps[:])

            def mk_epilogue(b, pair, pv0, pv1, sums_sb):
                def ep():
                    bcA = psS.tile([P, 512], f32, name="bcA", tag="ps_s",
                                   bufs=4)
                    nc.tensor.matmul(bcA[:], ones_row[:], sums_sb[:, 0:512],
                                     start=True, stop=True)
                    bcB = psS.tile([P, 512], f32, name="bcB", tag="ps_s",
                                   bufs=4)
                    nc.tensor.matmul(bcB[:], ones_row[:],
                                     sums_sb[:, 512:1024],
                                     start=True, stop=True)
                    rec = smallp.tile([P, 1024], f32, name="rec", tag="rec")
                    nc.vector.reciprocal_approx_fast(out=rec[:, 0:512],
                                                     in_=bcA[:])
                    nc.vector.reciprocal_approx_fast(out=rec[:, 512:1024],
                                                     in_=bcB[:])
                    attn = smallp.tile([P, 1024], f16, name="attn",
                                       tag="attn")
                    nc.vector.tensor_tensor(out=attn[:, 0:512], in0=pv0[:],
                                            in1=rec[:, 0:512], op=MULT)
                    nc.vector.tensor_tensor(out=attn[:, 512:1024],
                                            in0=pv1[:],
                                            in1=rec[:, 512:1024], op=MULT)
                    for jj in range(W):
                        nc.sync.dma_start(a2a_in[b][pair][jj, :, :],
                                          attn[:, jj * P:(jj + 1) * P])
                    nc.gpsimd.collective_compute(
                        "AllToAll", mybir.AluOpType.bypass,
                        replica_groups=[list(range(W))],
                        ins=[a2a_in[b][pair].opt()],
                        outs=[a2a_out[b][pair].opt()],
                    )
                    for i in range(W):
                        at = atpool.tile([P, P], f16, name=f"at{b}{pair}_{i}",
                                         tag=f"at{b}{pair}_{i}")
                        nc.gpsimd.dma_start(at[:], a2a_out[b][pair][i, :, :])
                        at_tiles[(b, pair, i)] = at
                return ep

            pending[0] = mk_epilogue(b, pair, pv0, pv1, sums_sb)
        pending[0]()
        pending[0] = None
        psPV.release()
        psS.release()

        # ---------------- Phase D: output projection ----------------
        # q-half qb only needs the (b, pair=qb) collectives; qb=0's
        # matmuls and epilogue overlap the final qb=1 collective.
        psD = tc.alloc_tile_pool(name="psD", bufs=1, space="PSUM")
        dpool = tc.alloc_tile_pool(name="dpool", bufs=1)
        with nc.named_scope("Dmm"):
            for qb in range(2):
                ps_y = [psD.tile([P, 512], f32, name=f"ps_y{qb}_{dc}",
                                 tag=f"ps_y{qb}_{dc}") for dc in range(4)]
                for b in range(NH):
                    for i in range(W):
                        at = at_tiles[(b, qb, i)]
                        h = 2 * i + b
                        start = (b == 0) and (i == 0)
                        stop = (b == NH - 1) and (i == W - 1)
                        for dc in range(4):
                            nc.tensor.matmul(
                                ps_y[dc][:], at[:],
                                wo_all[:, h, dc * 512:(dc + 1) * 512],
                                start=start, stop=stop)
                y_sb = dpool.tile([P, D], f16, name="y_sb", tag=f"y_sb{qb}")
                eng = nc.sync if qb == 0 else nc.scalar
                for dc in range(4):
                    dst = y_sb[:, dc * 512:(dc + 1) * 512]
                    if dc % 2 == 0:
                        nc.vector.tensor_copy(out=dst, in_=ps_y[dc][:])
                    else:
                        nc.scalar.copy(dst, ps_y[dc][:])
                    # chase each copy with its slice's DMA
                    eng.dma_start(y[qb * P:(qb + 1) * P,
                                    dc * 512:(dc + 1) * 512], dst)
        dpool.release()
        psD.release()

        for p in [atpool, smallp, prpool, espool, maskp, dram, consts,
                  persist]:
            p.release()

    nc.compile()
    return nc


def _warr(w, scale=None):
    """[256, D] -> [NH, P, DT, HD] host layout (contiguous per head)."""
    wt = w.T if scale is None else (w * scale).T          # [D, 256]
    a = wt.reshape(DT, P, NH, HD).transpose(2, 1, 0, 3)   # [NH, P, DT, HD]
    return np.ascontiguousarray(a)


_QPERM = np.array([(i % 1024) // P * QS + (i // 1024) * P + i % P
                   for i in range(S)])


def make_in_maps(x, mask, wq, wk, wv, wo):
    """x [1,S,D]; mask [1,16,S,S]; w* [D,D] (all f32) -> per-core dicts."""
    import ml_dtypes
    bf = ml_dtypes.bfloat16
    scale = np.float32(1.0 / math.sqrt(HD))
    xtv = np.ascontiguousarray(x[0].T).astype(bf)
    wo_tv = np.ascontiguousarray(wo.T).astype(np.float16)
    in_maps = []
    for c in range(W):
        rows = slice(NH * HD * c, NH * HD * (c + 1))
        m = mask[0, NH * c:NH * (c + 1)]
        emask = np.exp(m.transpose(0, 2, 1))[:, :, _QPERM].astype(np.float16)
        in_maps.append({
            "xt": xtv,
            "wq_t": _warr(wq[rows], scale).astype(bf),
            "wk_t": _warr(wk[rows]).astype(bf),
            "wv_t": _warr(wv[rows]).astype(bf),
            "emask_t": np.ascontiguousarray(emask),
            "wo_t": wo_tv,
        })
    return in_maps


def assemble(results):
    return np.concatenate(
        [results[c]["y"].astype(np.float32) for c in range(W)], axis=0)[None]


# ----------------------------------------------------------------------
# Harness entry point: kernel(**inputs) takes the FULL unsharded inputs
# as produced by setup_inputs() and returns the FULL [1, S, D] output.
# Inside: inputs are sharded head-wise (TP) across the 8 NeuronCores,
# the Bass kernel runs SPMD (with four AllToAll collectives), and the
# seq-sharded outputs are concatenated on the host.
# ----------------------------------------------------------------------
_NC_CACHE = []


def kernel(x, mask, start_pos, wq, wk, wv, wo):
    from concourse import bass_utils
    x = np.asarray(x, dtype=np.float32)
    mask = np.asarray(mask, dtype=np.float32)
    wq = np.asarray(wq, dtype=np.float32)
    wk = np.asarray(wk, dtype=np.float32)
    wv = np.asarray(wv, dtype=np.float32)
    wo = np.asarray(wo, dtype=np.float32)
    # start_pos == 0 prefill (as in the reference)
    if not _NC_CACHE:
        _NC_CACHE.append(build())
    nc = _NC_CACHE[0]
    in_maps = make_in_maps(x, mask, wq, wk, wv, wo)
    res = bass_utils.run_bass_kernel_spmd(nc, in_maps, core_ids=list(range(W)))
    return assemble(res.results).astype(np.float32)


# revision 20
# speedup vs baseline: 1.0223x; 1.0223x over previous
"""Tensor-parallel attention kernel for trn2 (8 cores).

TP over heads (2/core) for QKV + attention; AllToAll reshards attention
output to seq-parallel; output projection seq-sharded (each core owns
256 output rows); host concatenates.

v4 layout/engine plan:
- x, wq/wk/wv, q/k/v in bf16 (halves HBM + SBUF); mask shipped as
  exp(mask) f16 so the mask "add" becomes a 2x-rate DVE f16 multiply
  after the exp.
- Phase A: per 512-col chunk, three pair-passes (q0q1 / k0k1 / v0v1)
  reuse each streamed x tile from SBUF, so x is read once and the PE
  never waits on x DMA. x rides the sync/scalar HW queues exclusively;
  w, wo and mask prefetches ride the gpsimd queue so they never
  stall x.
- V transposed to [kv, dh] on the PE (identity matmul), deferred one
  chunk so the transposes never stall. (DMA-XBAR transposes complete
  out-of-order against regular copies on the shared HW DMA queues and
  corrupt results intermittently — do not use them here.)
- Attention q-columns are processed in a core-striped order (the
  mask is host-permuted to match): each half-pair covers 128 columns
  of EVERY core's 256-row output block, so an AllToAll of that half
  can launch as soon as the half finishes -> 4 small collectives that
  pipeline behind compute instead of 2 exposed ones.
- Attention per kv-tile: 2 score matmuls -> exp on Act -> DVE multiply
  by exp(mask) -> PV matmuls. Softmax denom: chunk B summed on PE
  (ones matmul accumulate), chunk A accumulated on DVE in f16 and
  reduced once. 1/sums via fast approx reciprocal. Per-pair epilogues
  are deferred into the next pair's pipeline so the PE never waits on
  the Act->PE sums round-trip.
- wo f16 fully prefetched into SBUF during phase A; output matmuls
  grouped by q-half so the last collective hides behind ready work;
  y written f16, widened on host.
"""
import math
import numpy as np

import concourse.bass as bass
import concourse.mybir as mybir
import concourse.tile as tile
from concourse import bacc

f32 = mybir.dt.float32
f32r = mybir.dt.float32r
f16 = mybir.dt.float16
bf16 = mybir.dt.bfloat16

P = 128
S = 2048
D = 2048
HD = 128
NH = 2          # heads per core
W = 8           # cores
QS = S // W     # 256 output rows per core
DT = D // P     # 16 contraction tiles
KT = S // P     # 16 kv tiles
CH = 4          # phase-A column chunks of 512

Exp = mybir.ActivationFunctionType.Exp
ADD = mybir.AluOpType.add
MULT = mybir.AluOpType.mult


def build():
    nc = bacc.Bacc("TRN2", target_bir_lowering=False, debug=False, num_devices=W)

    xt = nc.dram_tensor("xt", [D, S], bf16, kind="ExternalInput").ap()
    wq_t = nc.dram_tensor("wq_t", [NH, P, DT, HD], bf16, kind="ExternalInput").ap()
    wk_t = nc.dram_tensor("wk_t", [NH, P, DT, HD], bf16, kind="ExternalInput").ap()
    wv_t = nc.dram_tensor("wv_t", [NH, P, DT, HD], bf16, kind="ExternalInput").ap()
    # emask_t is host-permuted along q: column p*1024 + j*128 + i holds
    # exp(mask)[kv, j*256 + p*128 + i]
    emask_t = nc.dram_tensor("emask_t", [NH, S, S], f16, kind="ExternalInput").ap()
    wo_t = nc.dram_tensor("wo_t", [D, D], f16, kind="ExternalInput").ap()
    y = nc.dram_tensor("y", [QS, D], f16, kind="ExternalOutput").ap()

    wqkv = {"q": wq_t, "k": wk_t, "v": wv_t}

    with tile.TileContext(nc) as tc:
        persist = tc.alloc_tile_pool(name="persist", bufs=1)
        consts = tc.alloc_tile_pool(name="consts", bufs=1)
        maskp = tc.alloc_tile_pool(name="maskp", bufs=2)
        dram = tc.alloc_tile_pool(name="dram", bufs=1, space="DRAM")

        from concourse.masks import make_identity
        ident_f = consts.tile([P, P], f32, name="ident_f")
        make_identity(nc, ident_f[:])
        ident = consts.tile([P, P], bf16, name="ident")
        nc.vector.tensor_copy(out=ident[:], in_=ident_f[:])
        ones_cf = consts.tile([P, 1], f32, name="ones_cf")
        nc.gpsimd.memset(ones_cf[:], 1.0)
        ones_col = consts.tile([P, 1], f16, name="ones_col")
        nc.vector.tensor_copy(out=ones_col[:], in_=ones_cf[:])
        ones_rf = consts.tile([1, P], f32, name="ones_rf")
        nc.gpsimd.memset(ones_rf[:], 1.0)
        ones_row = consts.tile([1, P], f16, name="ones_row")
        nc.vector.tensor_copy(out=ones_row[:], in_=ones_rf[:])

        qT = [persist.tile([P, S], bf16, name=f"qT{b}") for b in range(NH)]
        kT = [persist.tile([P, S], bf16, name=f"kT{b}") for b in range(NH)]
        vT = [persist.tile([P, S], bf16, name=f"vT{b}") for b in range(NH)]
        v_sb = [persist.tile([P, KT, P], bf16, name=f"v_sb{b}") for b in range(NH)]
        # wo rows h*128:(h+1)*128 live at wo_all[:, h, :]
        wo_all = persist.tile([P, DT, D], f16, name="wo_all")

        # one AllToAll per (head, q-half): [W, dh, 128]
        a2a_in = [[dram.tile([W, P, P], f16, name=f"a2a_in{b}_{p}")
                   for p in range(2)] for b in range(NH)]
        a2a_out = [[dram.tile([W, P, P], f16, name=f"a2a_out{b}_{p}")
                    for p in range(2)] for b in range(NH)]

        def em_dma(b, pair, t):
            emt = maskp.tile([P, 1024], f16, name=f"em{t}", tag=f"em{t}", bufs=2)
            nc.gpsimd.dma_start(
                emt[:], emask_t[b, t * P:(t + 1) * P,
                                pair * 1024:(pair + 1) * 1024])
            return emt

        # ---------------- Phase A: projections ----------------
        psA = tc.alloc_tile_pool(name="psA", bufs=4, space="PSUM")
        wpool = tc.alloc_tile_pool(name="wpool", bufs=1)
        xpool = tc.alloc_tile_pool(name="xpool", bufs=1)

        kinds6 = [("q", 0), ("q", 1), ("k", 0), ("k", 1), ("v", 0), ("v", 1)]
        dests = {"q": qT, "k": kT, "v": vT}
        copy_eng = {("q", 0): "act", ("q", 1): "act",
                    ("k", 0): "dve", ("k", 1): "dve",
                    ("v", 0): "act", ("v", 1): "dve"}

        with nc.named_scope("A"):
            wtiles = {}
            for kind, b in kinds6:
                wt = wpool.tile([P, DT, HD], bf16, name=f"w_{kind}{b}",
                                tag=f"w_{kind}{b}")
                if kind == "q":
                    # split so the first matmuls only wait on 256KB
                    nc.gpsimd.dma_start(wt[:, 0:4, :], wqkv[kind][b][:, 0:4, :])
                    nc.gpsimd.dma_start(wt[:, 4:16, :],
                                        wqkv[kind][b][:, 4:16, :])
                else:
                    nc.gpsimd.dma_start(wt[:], wqkv[kind][b])
                wtiles[(kind, b)] = wt

            def do_transposes(c):
                for b in range(NH):
                    for t in range(4 * c, 4 * c + 4):
                        pst = psA.tile([P, P], bf16, name="pst", tag="pst",
                                       bufs=2)
                        nc.tensor.transpose(pst[:],
                                            vT[b][:, t * P:(t + 1) * P],
                                            ident[:])
                        nc.vector.tensor_copy(out=v_sb[b][:, t, :],
                                              in_=pst[:])

            def do_chunk(c):
                col0 = c * 512
                xts = []
                for t in range(DT):
                    xtile = xpool.tile([P, 512], bf16, name=f"xt{t}",
                                       tag=f"xt{t}")
                    (nc.sync if t % 2 == 0 else nc.scalar).dma_start(
                        xtile[:], xt[t * P:(t + 1) * P, col0:col0 + 512])
                    xts.append(xtile)
                if c > 0:
                    # previous chunk's V transposes: their PSUM->SBUF v
                    # copies have had a full chunk to land, so no PE stall
                    do_transposes(c - 1)
                for kind in ("q", "k", "v"):
                    pss = {}
                    for b in range(NH):
                        pss[b] = psA.tile([P, 512], f32,
                                          name=f"psa_{kind}{b}", tag="psa")
                    for t in range(DT):
                        for b in range(NH):
                            nc.tensor.matmul(pss[b][:],
                                             wtiles[(kind, b)][:, t, :],
                                             xts[t][:],
                                             start=(t == 0), stop=(t == DT - 1))
                    for b in range(NH):
                        dst = dests[kind][b][:, col0:col0 + 512]
                        if copy_eng[(kind, b)] == "act":
                            nc.scalar.copy(dst, pss[b][:])
                        else:
                            nc.vector.tensor_copy(out=dst, in_=pss[b][:])
                # wo prefetch rides the gpsimd queue (never stalls x);
                # the last rows go on sync at A's end so the first masks
                # aren't queued behind them
                if c < 3:
                    for h in range(4 * c, 4 * c + 4):
                        nc.gpsimd.dma_start(wo_all[:, h, :],
                                            wo_t[h * P:(h + 1) * P, :])

            for c in range(CH):
                do_chunk(c)
                if c == 2:
                    # first attention pair's masks stream during late A
                    em_next = [em_dma(0, 0, t) for t in range(KT)]
            do_transposes(CH - 1)
            for h in range(12, 16):
                nc.sync.dma_start(wo_all[:, h, :],
                                  wo_t[h * P:(h + 1) * P, :])

        xpool.release()
        wpool.release()
        psA.release()

        # ---------------- Phase B: attention ----------------
        psS = tc.alloc_tile_pool(name="psS", bufs=1, space="PSUM")
        psPV = tc.alloc_tile_pool(name="psPV", bufs=1, space="PSUM")
        espool = tc.alloc_tile_pool(name="espool", bufs=3)
        prpool = tc.alloc_tile_pool(name="prpool", bufs=3)
        smallp = tc.alloc_tile_pool(name="smallp", bufs=2)
        atpool = tc.alloc_tile_pool(name="atpool", bufs=1)

        at_tiles = {}
        pairs = [(b, p) for b in range(NH) for p in range(2)]
        pending = [None]  # deferred epilogue closure

        for pi, (b, pair) in enumerate(pairs):
          with nc.named_scope(f"B{b}{pair}"):
            emts = em_next
            if pi + 1 < len(pairs):
                nb, np_ = pairs[pi + 1]
                em_next = [em_dma(nb, np_, t) for t in range(KT)]
            # core-striped q columns: new col p*1024 + j*128 + i reads
            # qT col j*256 + p*128 + i  (emask already host-permuted)
            qr = qT[b][:].rearrange("d (j i) -> d j i", i=QS)
            qA = qr[:, 0:4, pair * P:(pair + 1) * P]
            qB = qr[:, 4:8, pair * P:(pair + 1) * P]
            pv0 = psPV.tile([P, 512], f32, name="pv0", tag="pv0")
            pv1 = psPV.tile([P, 512], f32, name="pv1", tag="pv1")
            sums_ps = psS.tile([1, 512], f32, name="sums_ps",
                               tag="sums", bufs=1)
            sacc = smallp.tile([P, 512], f16, name="sacc", tag="sacc")
            PIPE = 3
            pq = {}
            for tt in range(KT + PIPE):
                if tt < KT:
                    t = tt
                    ktile = kT[b][:, t * P:(t + 1) * P]
                    ps_sA = psS.tile([P, 512], f32, name="ps_sA",
                                     tag="ps_s", bufs=4)
                    nc.tensor.matmul(ps_sA[:], ktile, qA,
                                     start=True, stop=True)
                    ps_sB = psS.tile([P, 512], f32, name="ps_sB",
                                     tag="ps_s", bufs=4)
                    nc.tensor.matmul(ps_sB[:], ktile, qB,
                                     start=True, stop=True)
                    esA = espool.tile([P, 512], f16, name="esA", tag="es", bufs=4)
                    nc.scalar.activation(esA[:], ps_sA[:], Exp)
                    esB = espool.tile([P, 512], f16, name="esB", tag="es", bufs=4)
                    nc.scalar.activation(esB[:], ps_sB[:], Exp)
                    prA = prpool.tile([P, 512], f16, name="prA", tag="pr", bufs=5)
                    nc.vector.tensor_tensor(out=prA[:], in0=esA[:],
                                            in1=emts[t][:, 0:512], op=MULT)
                    prB = prpool.tile([P, 512], f16, name="prB", tag="pr", bufs=5)
                    nc.vector.tensor_tensor(out=prB[:], in0=esB[:],
                                            in1=emts[t][:, 512:1024], op=MULT)
                    pq[t] = (prA, prB)
                if tt == 1 and pending[0] is not None:
                    # previous pair's deferred epilogue slots in here so
                    # its PE work overlaps this pair's pipeline fill
                    pending[0]()
                    pending[0] = None
                if tt >= PIPE:
                    t = tt - PIPE
                    prA, prB = pq.pop(t)
                    vtile = v_sb[b][:, t, :]
                    nc.tensor.matmul(pv0[:], vtile, prA[:],
                                     start=(t == 0), stop=(t == KT - 1))
                    nc.tensor.matmul(pv1[:], vtile, prB[:],
                                     start=(t == 0), stop=(t == KT - 1))
                    nc.tensor.matmul(sums_ps[:], ones_col[:], prB[:],
                                     start=(t == 0), stop=(t == KT - 1))
                    if t == 0:
                        nc.vector.tensor_copy(out=sacc[:], in_=prA[:])
                    else:
                        nc.vector.tensor_tensor(out=sacc[:], in0=sacc[:],
                                                in1=prA[:], op=ADD)
            # part 1 of epilogue: fold chunk-A sums, copy both to SBUF
            sumsA_ps = psS.tile([1, 512], f32, name="sumsA_ps",
                                tag="sumsA", bufs=1)
            nc.tensor.matmul(sumsA_ps[:], ones_col[:], sacc[:],
                             start=True, stop=True)
            sums_sb = smallp.tile([1, 1024], f16, name="sums_sb",
                                  tag="sums_sb")
            nc.scalar.copy(sums_sb[:, 0:512], sumsA_ps[:])
            nc.scalar.copy(sums_sb[:, 512:1024], sums_ps[:])

            def mk_epilogue(b, pair, pv0, pv1, sums_sb):
                def ep():
                    bcA = psS.tile([P, 512], f32, name="bcA", tag="ps_s",
                                   bufs=4)
                    nc.tensor.matmul(bcA[:], ones_row[:], sums_sb[:, 0:512],
                                     start=True, stop=True)
                    bcB = psS.tile([P, 512], f32, name="bcB", tag="ps_s",
                                   bufs=4)
                    nc.tensor.matmul(bcB[:], ones_row[:],
                                     sums_sb[:, 512:1024],
                                     start=True, stop=True)
                    rec = smallp.tile([P, 1024], f32, name="rec", tag="rec")
                    nc.vector.reciprocal_approx_fast(out=rec[:, 0:512],
                                                     in_=bcA[:])
                    nc.vector.reciprocal_approx_fast(out=rec[:, 512:1024],
                                                     in_=bcB[:])
                    attn = smallp.tile([P, 1024], f16, name="attn",
                                       tag="attn")
                    nc.vector.tensor_tensor(out=attn[:, 0:512], in0=pv0[:],
                                            in1=rec[:, 0:512], op=MULT)
                    nc.vector.tensor_tensor(out=attn[:, 512:1024],
                                            in0=pv1[:],
                                            in1=rec[:, 512:1024], op=MULT)
                    for jj in range(W):
                        nc.sync.dma_start(a2a_in[b][pair][jj, :, :],
                                          attn[:, jj * P:(jj + 1) * P])
                    nc.gpsimd.collective_compute(
                        "AllToAll", mybir.AluOpType.bypass,
                        replica_groups=[list(range(W))],
                        ins=[a2a_in[b][pair].opt()],
                        outs=[a2a_out[b][pair].opt()],
                    )
                    for i in range(W):
                        at = atpool.tile([P, P], f16, name=f"at{b}{pair}_{i}",
                                         tag=f"at{b}{pair}_{i}")
                        nc.gpsimd.dma_start(at[:], a2a_out[b][pair][i, :, :])
                        at_tiles[(b, pair, i)] = at
                return ep

            pending[0] = mk_epilogue(b, pair, pv0, pv1, sums_sb)
        pending[0]()
        pending[0] = None
        psPV.release()
        psS.release()

        # ---------------- Phase D: output projection ----------------
        # q-half qb only needs the (b, pair=qb) collectives; qb=0's
        # matmuls and epilogue overlap the final qb=1 collective.
        psD = tc.alloc_tile_pool(name="psD", bufs=1, space="PSUM")
        dpool = tc.alloc_tile_pool(name="dpool", bufs=1)
        with nc.named_scope("Dmm"):
            for qb in range(2):
                ps_y = [psD.tile([P, 512], f32, name=f"ps_y{qb}_{dc}",
                                 tag=f"ps_y{qb}_{dc}") for dc in range(4)]
                for b in range(NH):
                    for i in range(W):
                        at = at_tiles[(b, qb, i)]
                        h = 2 * i + b
                        start = (b == 0) and (i == 0)
                        stop = (b == NH - 1) and (i == W - 1)
                        for dc in range(4):
                            nc.tensor.matmul(
                                ps_y[dc][:], at[:],
                                wo_all[:, h, dc * 512:(dc + 1) * 512],
                                start=start, stop=stop)
                y_sb = dpool.tile([P, D], f16, name="y_sb", tag=f"y_sb{qb}")
                eng = nc.sync if qb == 0 else nc.scalar
                for dc in range(4):
                    dst = y_sb[:, dc * 512:(dc + 1) * 512]
                    if dc % 2 == 0:
                        nc.vector.tensor_copy(out=dst, in_=ps_y[dc][:])
                    else:
                        nc.scalar.copy(dst, ps_y[dc][:])
                    # chase each copy with its slice's DMA
                    eng.dma_start(y[qb * P:(qb + 1) * P,
                                    dc * 512:(dc + 1) * 512], dst)
        dpool.release()
        psD.release()

        for p in [atpool, smallp, prpool, espool, maskp, dram, consts,
                  persist]:
            p.release()

    nc.compile()
    return nc


def _warr(w, scale=None):
    """[256, D] -> [NH, P, DT, HD] host layout (contiguous per head)."""
    wt = w.T if scale is None else (w * scale).T          # [D, 256]
    a = wt.reshape(DT, P, NH, HD).transpose(2, 1, 0, 3)   # [NH, P, DT, HD]
    return np.ascontiguousarray(a)


_QPERM = np.array([(i % 1024) // P * QS + (i // 1024) * P + i % P
                   for i in range(S)])


def make_in_maps(x, mask, wq, wk, wv, wo):
    """x [1,S,D]; mask [1,16,S,S]; w* [D,D] (all f32) -> per-core dicts."""
    import ml_dtypes
    bf = ml_dtypes.bfloat16
    scale = np.float32(1.0 / math.sqrt(HD))
    xtv = np.ascontiguousarray(x[0].T).astype(bf)
    wo_tv = np.ascontiguousarray(wo.T).astype(np.float16)
    in_maps = []
    for c in range(W):
        rows = slice(NH * HD * c, NH * HD * (c + 1))
        m = mask[0, NH * c:NH * (c + 1)]
        emask = np.exp(m.transpose(0, 2, 1))[:, :, _QPERM].astype(np.float16)
        in_maps.append({
            "xt": xtv,
            "wq_t": _warr(wq[rows], scale).astype(bf),
            "wk_t": _warr(wk[rows]).astype(bf),
            "wv_t": _warr(wv[rows]).astype(bf),
            "emask_t": np.ascontiguousarray(emask),
            "wo_t": wo_tv,
        })
    return in_maps


def assemble(results):
    return np.concatenate(
        [results[c]["y"].astype(np.float32) for c in range(W)], axis=0)[None]


# ----------------------------------------------------------------------
# Harness entry point: kernel(**inputs) takes the FULL unsharded inputs
# as produced by setup_inputs() and returns the FULL [1, S, D] output.
# Inside: inputs are sharded head-wise (TP) across the 8 NeuronCores,
# the Bass kernel runs SPMD (with four AllToAll collectives), and the
# seq-sharded outputs are concatenated on the host.
# ----------------------------------------------------------------------
_NC_CACHE = []


def kernel(x, mask, start_pos, wq, wk, wv, wo):
    from concourse import bass_utils
    x = np.asarray(x, dtype=np.float32)
    mask = np.asarray(mask, dtype=np.float32)
    wq = np.asarray(wq, dtype=np.float32)
    wk = np.asarray(wk, dtype=np.float32)
    wv = np.asarray(wv, dtype=np.float32)
    wo = np.asarray(wo, dtype=np.float32)
    # start_pos == 0 prefill (as in the reference)
    if not _NC_CACHE:
        _NC_CACHE.append(build())
    nc = _NC_CACHE[0]
    in_maps = make_in_maps(x, mask, wq, wk, wv, wo)
    res = bass_utils.run_bass_kernel_spmd(nc, in_maps, core_ids=list(range(W)))
    return assemble(res.results).astype(np.float32)
